# revision 60
# baseline (speedup 1.0000x reference)
"""DescriptorRetentionLoss on 8 Trainium2 cores — per-tile window version.

Matches are within 2px in a 640x480 image, so they are spatially local in x.
The host sorts both point sets by x; core c gets the c-th x-strip of 512
valid rows.  Within a core, each 128-row tile t spans only ~20px, so its
candidate memory columns (strip +-3px) fit in a WT=416 window of the sorted
memory arrays (measured max 395).  Windows are real column slices (clamped,
padded with real neighbors), so every computed quantity is exact.

Per (core, tile): a 15-row fp16 split-product matmul reproduces the
reference's fp32 d2 to ~fp32-accumulation accuracy (all split pieces have
<=11-bit mantissas, so every PE product is exact); tiles 0,1 threshold on the
DVE (is_lt, 0/1 masks), tiles 2,3 on the Act engine (Sign, +-1 masks, de-
signed on the host with a tiny x.z window-sum correction for the masked-
cosine term).  The fp8e4m3 DoubleRow descriptor matmul gives x . yhat*16 and
one affine_mul_reduce per tile accumulates mc_i = sum_m mask * cos.  A
[4, WT] PSUM tile accumulates per-tile column counts via one-hot weights;
counts are integers |v|<=128, shipped bf16.  A short dummy-matmul stream
keeps the PE p-state ramped through the input-DMA window.

Everything quadratic stays on device; the host finishes with O((N+M)*D):
  loss = (S@T + A) / (max(n_pairs,1) * max(n_rows,1))
    S: scatter-add of per-tile column counts into the global [M] array
    T[m] = W . yhat_m with W = sum_i [rc_i>0] * x_i/|x_i|  (from rc_out)
    A = sum(rc) - 2*sum(mc)

Inputs are packed into three DMA-friendly blobs (pts, desc8a, desc8b) spread
over the SP and Act DGE queues to minimize the serialized load chain.
"""

import sys

sys.path.insert(0, "/opt/trn_rl_repo")

import numpy as np
from contextlib import ExitStack

N, M, D = 4096, 8192, 512
NCORES = 8
NL = N // NCORES          # 512 rows per core
NT = NL // 128            # 4 row tiles
KC = D // 128             # 4 contraction chunks
WT = 416                  # per-tile candidate window (measured max 395)
MARGIN = 3.0              # px window margin; safe vs fp32 d2 rounding
PTSW1 = 2 * WT + NL       # pts1 width: windows 0,1 + xpts
PTSW2 = 2 * WT            # pts2 width: windows 2,3
DAW = NL + WT             # desc8a per-chunk width (x + window t=0)
DA2W = WT                 # desc8a2 per-chunk width (window t=1)
DBW = 2 * WT + 2          # desc8b per-chunk width (windows t=2,3 + z cols)
NSGN = 2                  # row tiles 2,3 use Act Sign masks (+-1 convention)

_cached = {}


def _split11(v):
    """Exact 2-piece split of fp32 into <=11-mantissa-bit halves."""
    v = np.asarray(v, np.float32)
    m, e = np.frexp(v)
    hi = np.ldexp(np.trunc(np.ldexp(m, 11)), e - 11).astype(np.float32)
    return hi, (v - hi).astype(np.float32)


def _split11_multi(v64, n):
    pieces = []
    rem = np.asarray(v64, np.float64)
    for _ in range(n):
        r32 = rem.astype(np.float32)
        m, e = np.frexp(r32)
        hi = np.ldexp(np.trunc(np.ldexp(m, 11)), e - 11).astype(np.float32)
        pieces.append(hi)
        rem = rem - hi.astype(np.float64)
    return pieces


def _split3(v):
    """Exact 3-piece split of fp32 into <=11-mantissa-bit halves (fp16-safe)."""
    h, r = _split11(v)
    m, l = _split11(r)
    return h, m, l


# per-coordinate (x-piece, y-piece) product pairs; (m,l),(l,m),(l,l) dropped
_PAIRS = [(0, 0), (0, 1), (1, 0), (0, 2), (2, 0), (1, 1)]
KROWS = 2 * len(_PAIRS) + 3   # 15


def _mk_xpts(xp):
    """[KROWS, n] fp16 x-side rows; row k pairs with _mk_ypts row k."""
    rows = []
    for j in range(2):
        pieces = _split3(xp[:, j])
        rows += [-2.0 * pieces[a] for a, _ in _PAIRS]
    sixteen = np.full(xp.shape[0], 16.0, np.float32)
    rows += [sixteen, sixteen, sixteen]
    return np.ascontiguousarray(np.stack(rows).astype(np.float16))


def _mk_ypts(yp):
    rows = []
    for j in range(2):
        pieces = _split3(yp[:, j])
        rows += [pieces[b] for _, b in _PAIRS]
    yy64 = (yp[:, 0].astype(np.float64) ** 2 + yp[:, 1].astype(np.float64) ** 2)
    yy1, yy2, yy3 = _split11_multi(yy64, 3)
    rows += [yy1 / 16.0, yy2 / 16.0, yy3 / 16.0]
    return np.ascontiguousarray(np.stack(rows).astype(np.float16))


def _f8():
    import ml_dtypes
    return ml_dtypes.float8_e4m3fn


def _build_nc():
    from concourse import bacc, mybir, tile

    f32 = mybir.dt.float32
    f16 = mybir.dt.float16
    bf16 = mybir.dt.bfloat16
    f8e4 = mybir.dt.float8e4
    nc = bacc.Bacc("TRN2", target_bir_lowering=False, debug=False)

    # pts1: [k, t*WT+m] = ypts windows 0,1; [k, 2*WT+n] = xpts.  pts2:
    # ypts windows 2,3.  fp16: every row is a <=11-bit-mantissa split piece,
    # exactly fp16-representable; the PE multiplies fp16 exactly into PSUM.
    pts1 = nc.dram_tensor("pts1", [KROWS, PTSW1], f16, kind="ExternalInput")
    pts2 = nc.dram_tensor("pts2", [KROWS, PTSW2], f16, kind="ExternalInput")
    # thrrx: row (0,t) = thr tile t, row (1,t) = rx16 tile t
    thrrx = nc.dram_tensor("thrrx", [2 * NT, 128], f32, kind="ExternalInput")
    # desc8a: [p, c, 0:NL] = xdT8; [p, c, NL+m] = y window 0
    desc8a = nc.dram_tensor("desc8a", [128, KC, DAW], f8e4, kind="ExternalInput")
    # desc8a2: [p, c, m] = y window 1
    desc8a2 = nc.dram_tensor("desc8a2", [128, KC, DA2W], f8e4,
                             kind="ExternalInput")
    # desc8b: [p, c, t*WT+m] = y window 2+t (t=0,1)
    desc8b = nc.dram_tensor("desc8b", [128, KC, DBW], f8e4, kind="ExternalInput")

    # per-tile column sums are integers in [-128, 128]: exact in bf16
    S_out = nc.dram_tensor("S_out", [NT, WT], bf16, kind="ExternalOutput")
    rc_out = nc.dram_tensor("rc_out", [NL], f32, kind="ExternalOutput")
    mc_out = nc.dram_tensor("mc_out", [128 * (NT + NSGN)], f32,
                            kind="ExternalOutput")

    AF = mybir.ActivationFunctionType
    OP = mybir.AluOpType
    DR = mybir.MatmulPerfMode.DoubleRow

    with ExitStack() as ctx:
        tc = ctx.enter_context(tile.TileContext(nc))
        singles = ctx.enter_context(tc.tile_pool(name="singles", bufs=1))
        small = ctx.enter_context(tc.tile_pool(name="small", bufs=8))
        evac_pool = ctx.enter_context(tc.tile_pool(name="evac", bufs=2))
        ps_p = ctx.enter_context(tc.tile_pool(name="ps_p", bufs=2, space="PSUM"))
        ps_d = ctx.enter_context(tc.tile_pool(name="ps_d", bufs=2, space="PSUM"))
        ps_s = ctx.enter_context(tc.tile_pool(name="ps_s", bufs=1, space="PSUM"))
        ps_w = ctx.enter_context(tc.tile_pool(name="ps_w", bufs=1, space="PSUM"))

        # one-hot column weights: eye[t] is [128, NT] bf16 with column t = 1
        eyes = []
        for t in range(NT):
            e = singles.tile([128, NT], bf16, name=f"eye{t}", tag=f"eye{t}")
            nc.vector.memset(e, 0.0)
            nc.vector.memset(e[:, t:t + 1], 1.0)
            eyes.append(e)

        # trigger the implicit activation-table load immediately so it does
        # not delay the Sign masks mid-stream
        actwarm = singles.tile([128, 1], bf16, name="actwarm", tag="aw")
        nc.scalar.copy(actwarm, eyes[0][:, 0:1])

        # keep the PE busy through the input-DMA window so the p-state clock
        # is fully ramped (>3us continuous) when the real matmuls arrive
        warm_src = singles.tile([128, WT], bf16, name="warmsrc", tag="ws")
        nc.vector.memset(warm_src, 0.0)
        warm_w = singles.tile([128, 128], bf16, name="warmw", tag="ww")
        nc.vector.memset(warm_w, 0.0)
        pwarm = ps_w.tile([128, WT], f32)
        for i in range(4):
            nc.tensor.matmul(pwarm, warm_w, warm_src, start=True, stop=True)

        # ---- packed loads spread over SP / Pool DGE queues ----
        spts1 = singles.tile([KROWS, PTSW1], f16)
        nc.sync.dma_start(out=spts1, in_=pts1[:, :])
        spts2 = singles.tile([KROWS, PTSW2], f16)
        nc.sync.dma_start(out=spts2, in_=pts2[:, :])
        strx = singles.tile([128, 2 * NT], f32)
        nc.gpsimd.dma_start(out=strx, in_=thrrx.rearrange("x p -> p x"))
        sda = singles.tile([128, KC, DAW], f8e4)
        nc.gpsimd.dma_start(out=sda, in_=desc8a[:, :, :])
        sda2 = singles.tile([128, KC, DA2W], f8e4)
        nc.sync.dma_start(out=sda2, in_=desc8a2[:, :, :])
        sdb = singles.tile([128, KC, DBW], f8e4)
        nc.gpsimd.dma_start(out=sdb, in_=desc8b[:, :, :])

        def syp(t):
            if t < 2:
                return spts1[:, t * WT:(t + 1) * WT]
            return spts2[:, (t - 2) * WT:(t - 1) * WT]

        def sxp(t):
            return spts1[:, 2 * WT + t * 128:2 * WT + (t + 1) * 128]

        def sxd(t, c0):
            return sda[:, c0:c0 + 2, t * 128:(t + 1) * 128]

        def syd(t, c0):
            if t == 0:
                return sda[:, c0:c0 + 2, NL:NL + WT]
            if t == 1:
                return sda2[:, c0:c0 + 2, :]
            return sdb[:, c0:c0 + 2, (t - 2) * WT:(t - 2) * WT + WT]

        sthr = strx[:, 0:NT]
        srx = strx[:, NT:2 * NT]

        rcst = singles.tile([128, NT], f32)
        # mcq: cols 0..NT-1 = mcst; cols NT..NT+1 = q (window-sum dots, t=2,3)
        mcq = singles.tile([128, NT + NSGN], f32)
        mf_all = singles.tile([128, NT, WT], bf16)
        pS4 = ps_s.tile([NT, WT], f32)

        # masks first: t<2 on DVE as 0/1 (is_lt); t>=2 on Act as +-1 (Sign).
        # Host converts the +-1 conventions back to counts.  t=2,3 lead: their
        # descriptor windows (sdb, Pool queue) land first, so their amr can
        # start while sda is still in flight.
        TORD = (2, 3, 0, 1)
        for t in TORD:
            pp = ps_p.tile([128, WT], f32, name=f"pp{t}", tag="pp")
            nc.tensor.matmul(pp, sxp(t), syp(t), start=True, stop=True)
            if t < NT - NSGN:
                nc.vector.tensor_scalar(
                    out=mf_all[:, t, :], in0=pp, scalar1=sthr[:, t:t + 1],
                    scalar2=None, op0=OP.is_lt, op1=OP.add,
                    accum_out=rcst[:, t:t + 1])
            else:
                nc.scalar.activation(
                    mf_all[:, t, :], pp, AF.Sign, bias=sthr[:, t:t + 1],
                    scale=-1.0, accum_out=rcst[:, t:t + 1])

        pds = {}
        for t in TORD:
            pd = ps_d.tile([128, WT], f32, name=f"pd{t}", tag="pd")
            nc.tensor.matmul(pd, sxd(t, 0), syd(t, 0), start=True, stop=False,
                             perf_mode=DR)
            nc.tensor.matmul(pd, sxd(t, 2), syd(t, 2), start=False, stop=True,
                             perf_mode=DR)
            pds[t] = pd

        # q_t[i] = x_i . z_t for the Sign-mask correction (z cols ride in sdb)
        pZ = ps_w.tile([128, NSGN], f32, name="pZ", tag="pZ")
        for j in range(NSGN):
            nc.tensor.matmul(pZ[:, j:j + 1],
                             sda[:, 0:2, (2 + j) * 128:(3 + j) * 128],
                             sdb[:, 0:2, DBW - 2 + j:DBW - 1 + j],
                             start=True, stop=False, perf_mode=DR)
            nc.tensor.matmul(pZ[:, j:j + 1],
                             sda[:, 2:4, (2 + j) * 128:(3 + j) * 128],
                             sdb[:, 2:4, DBW - 2 + j:DBW - 1 + j],
                             start=False, stop=True, perf_mode=DR)
        nc.scalar.activation(mcq[:, NT:NT + NSGN], pZ, AF.Copy)

        nc.sync.dma_start(out=rc_out.rearrange("(p t) -> p t", p=128), in_=rcst)

        for t in TORD:
            dummy = small.tile([128, 1], f32, name=f"dm{t}", tag="dm")
            nc.vector.affine_mul_reduce(
                out=dummy.broadcast_to(pds[t].shape),
                accum_out=mcq[:, t:t + 1],
                in0=pds[t], in1=mf_all[:, t, :], scale=srx[:, t:t + 1], bias=0.0)

        for t in TORD:
            nc.tensor.matmul(pS4, eyes[t], mf_all[:, t, :], start=(t == TORD[0]),
                             stop=(t == TORD[-1]))
        sS4 = evac_pool.tile([NT, WT], bf16, name="sS4", tag="sS")
        nc.scalar.activation(sS4, pS4, AF.Copy)
        nc.scalar.dma_start(out=S_out[:, :], in_=sS4)

        nc.sync.dma_start(out=mc_out.rearrange("(p x) -> p x", p=128), in_=mcq)

    nc.finalize()
    return nc


def _get_nc():
    if "nc" not in _cached:
        _cached["nc"] = _build_nc()
    return _cached["nc"]


def _make_in_maps(valid_pts_scr, mem_pts_scr, valid_desc, mem_desc):
    """Returns (in_maps, meta); meta carries window offsets + host y/x data."""
    f8 = _f8()
    vp = np.asarray(valid_pts_scr, np.float32)
    mp = np.asarray(mem_pts_scr, np.float32)
    vd = np.asarray(valid_desc, np.float32)
    md = np.asarray(mem_desc, np.float32)

    xs = np.argsort(vp[:, 0], kind="stable")
    ms = np.argsort(mp[:, 0], kind="stable")
    vp_s, vd_s = vp[xs], vd[xs]
    mp_s, md_s = mp[ms], md[ms]

    yy = np.sum(md_s.astype(np.float64) ** 2, -1)
    ry = 1.0 / np.sqrt(yy)
    yhat = (md_s * ry[:, None]).astype(np.float32)                 # [M, D]
    # [KC, 128, M] view: ydT8_c[c, p, m] = yhat16[col m, d = c*128 + p]
    ydT8_full = (yhat * 16.0).T.astype(f8).reshape(KC, 128, M)
    ypts_full = _mk_ypts(mp_s)                                     # [11, M]

    xx_pt = (vp_s[:, 0].astype(np.float64) ** 2
             + vp_s[:, 1].astype(np.float64) ** 2)
    xx_d = np.sum(vd_s.astype(np.float64) ** 2, -1)
    rx = 1.0 / np.sqrt(xx_d)

    in_maps, offs = [], []
    for c in range(NCORES):
        rows = slice(c * NL, (c + 1) * NL)
        xp = vp_s[rows]
        pts1 = np.empty((KROWS, PTSW1), np.float16)
        pts2 = np.empty((KROWS, PTSW2), np.float16)
        pts1[:, 2 * WT:] = _mk_xpts(xp)
        desc8a = np.empty((128, KC, DAW), f8)
        desc8a2 = np.empty((128, KC, DA2W), f8)
        desc8a[:, :, :NL] = (vd_s[rows].T.astype(f8)
                             .reshape(KC, 128, NL).transpose(1, 0, 2))
        desc8b = np.empty((128, KC, DBW), f8)
        offs_c = []
        for t in range(NT):
            tp = xp[t * 128:(t + 1) * 128]
            lo = tp[:, 0].min() - MARGIN
            hi = tp[:, 0].max() + MARGIN
            s = int(np.searchsorted(mp_s[:, 0], lo, side="left"))
            e = int(np.searchsorted(mp_s[:, 0], hi, side="right"))
            assert e - s <= WT, f"core {c} tile {t}: window {e - s} > WT {WT}"
            s = min(s, M - WT)
            offs_c.append(s)
            ypw = ypts_full[:, s:s + WT]
            if t < 2:
                pts1[:, t * WT:(t + 1) * WT] = ypw
            else:
                pts2[:, (t - 2) * WT:(t - 1) * WT] = ypw
            ywin = ydT8_full[:, :, s:s + WT].transpose(1, 0, 2)  # [128, KC, WT]
            if t == 0:
                desc8a[:, :, NL:NL + WT] = ywin
            elif t == 1:
                desc8a2[:, :, :] = ywin
            else:
                desc8b[:, :, (t - 2) * WT:(t - 2) * WT + WT] = ywin
                # z column: per-d sum of the fp8 window values (for the
                # Sign-mask rowsum correction)
                z32 = ywin.astype(np.float32).sum(axis=2)  # [128, KC]
                desc8b[:, :, DBW - 2 + (t - 2)] = z32.astype(f8)
        offs.append(offs_c)

        thrrx = np.empty((2 * NT, 128), np.float32)
        thrrx[0:NT] = (4.0 - xx_pt[rows]).astype(np.float32).reshape(NT, 128)
        thrrx[NT:] = (rx[rows] / 16.0).astype(np.float32).reshape(NT, 128)

        in_maps.append({
            "pts1": pts1,
            "pts2": pts2,
            "thrrx": thrrx,
            "desc8a": desc8a,
            "desc8a2": desc8a2,
            "desc8b": desc8b,
        })
    meta = {"offs": offs, "yhat": yhat, "vd_s": vd_s, "rx": rx}
    return in_maps, meta


def _finish(results, meta):
    offs = meta["offs"]
    yhat = meta["yhat"]
    rx = meta["rx"]
    Stot = np.zeros(M, np.float64)
    A = 0.0
    nrows = 0.0
    rh_all = np.zeros(N, np.float64)
    nsk = NT - NSGN  # first tile using the +-1 Sign convention
    for c in range(NCORES):
        r = results[c]
        S4 = r["S_out"].astype(np.float64)
        rc_raw = r["rc_out"].astype(np.float64).reshape(128, NT)
        mcq = r["mc_out"].astype(np.float64).reshape(128, NT + NSGN)
        for t in range(NT):
            s = offs[c][t]
            St = S4[t] if t < nsk else (S4[t] + 128.0) / 2.0
            Stot[s:s + WT] += St
            rows = slice(c * NL + t * 128, c * NL + (t + 1) * 128)
            rc_t = rc_raw[:, t] if t < nsk else (rc_raw[:, t] + WT) / 2.0
            if t < nsk:
                mc_t = mcq[:, t]
            else:
                q = mcq[:, NT + (t - nsk)]
                mc_t = (mcq[:, t] + (rx[rows] / 16.0) * q) / 2.0
            A += rc_t.sum() - 2.0 * mc_t.sum()
            nrows += float((rc_t > 0.5).sum())
            rh_all[rows] = rc_t > 0.5
    npairs = Stot.sum()
    if nrows > 0:
        W = ((rh_all * rx)[:, None] * meta["vd_s"]).sum(0)  # [D]
        T = yhat.astype(np.float64) @ W
        loss = (Stot @ T + A) / (max(npairs, 1.0) * max(nrows, 1.0))
    else:
        loss = 0.0
    return np.float32(loss)


def kernel(valid_pts_scr, mem_pts_scr, valid_desc, mem_desc):
    from concourse.bass_utils import run_bass_kernel_spmd

    in_maps, meta = _make_in_maps(valid_pts_scr, mem_pts_scr,
                                  valid_desc, mem_desc)
    nc = _get_nc()
    res = run_bass_kernel_spmd(nc, in_maps, core_ids=list(range(NCORES)))
    _cached["last_results"] = res
    return _finish(res.results, meta)
